# revision 49
# baseline (speedup 1.0000x reference)
"""Trainium2 Bass kernel for nn_Attention (B=32, P=577, D=768, 12 heads).

Strategy: data-parallel over batch — 4 batch elements per core on 8 cores,
zero collectives. Projections and scores run in transposed ([feature,
token]) layouts; attention-weighted V runs q-major and is PE-transposed
back:

  xT [768,2308] --(wqkvT)--> qkT [1536,577/batch] (e on partitions)
                         `--> V   [577/batch,768]  (token on partitions)
  per (b,h): ST = K Q^T   (ST[p,q], K=64, head pairs in PE row halves)
             AT = exp(scale*ST)        (ScalarE, softmax w/o max-subtract:
                                        |scale*S| < ~2, exp is safe)
  O[q, s|1] = AT^T [V|1]  (q-major; ones column of V lands the softmax
             denominator at col 64; all five q-tiles accumulate in a
             single-bank PSUM tile [128, 5, 65])
  O_n = O * (1/denom)     (DVE reciprocal + per-partition scalar multiply —
             no partition broadcast, no partition-shift DMA)
  ot[(b,j2)] = PE-transpose of head-pair O_n  (identity-matmul, bf16 PSUM)
  yT [768,577/batch] = w_outT^T ot + b_eff (v-bias folded into b_eff on host)

Scheduling: the PE executes in emission order, so the emitter software-
pipelines at sub-batch granularity; per (b, j2) iteration the five ST score
units (each gated by the previous exp's PSUM drain) are spaced by
always-ready filler matmuls:

  ST(qt0) PROJV ST(qt1) PROJ ST(qt2) AVQ OUT ST(qt3) AVQ ST(qt4) PROJ TPOSE

(OUT rides the ST2->ST3 gap since the q-major AVQ unit alone is shorter
than the exp cadence — at j2=0 it follows TPOSE(b-1,5) instead; the PROJ
filler precedes TPOSE so the DVE normalize-multiplies land before the PE
transposes read them.)

Every batch (including the last) carries its own next projection pair as
filler, so no iteration is starved. The q-major AV keeps the whole
normalization on DVE with a short PSUM drain, which recycles the shared
PSUM ring quickly — this removed ~160us/rep of real-HW cross-engine stall
versus the O^T formulation with gpsimd partition-broadcast.

Startup: the first ST gates on six q/k weight halves + batch-0 x, so those
lead both HWDGE queues (4 w-qk on SP, 2 on ACT behind the x chunks);
biases, v-weights, out-weights and the remaining x batches follow. Output
is stored bf16 (halves output DMA; converted to f32 on host). Inputs are
bf16; PSUM accumulation is f32.

Measured (axon trn2, differential reps=10 interleaved block-medians):
~239us/exec (rounds 210-292us) vs ~475us for the O^T checkpoint and
~465-495us for the original baseline under equivalent conditions — the
q-major rewrite measured ~164us/rep faster than the checkpoint in matched
A/B, and the PROJ-before-TPOSE tail reorder a further ~70us. TimelineSim
predicts 300us; real HW amplifies cross-engine stall costs, so emission-
order fixes often pay more than the simulator suggests.
"""

import numpy as np
import ml_dtypes

import concourse.bass as bass
import concourse.tile as tile
from concourse import bacc, mybir
from concourse.bass_utils import run_bass_kernel_spmd
from concourse.masks import make_identity

# problem dims (hardcoded per harness contract)
B, PL, D = 32, 577, 768
H, S = 12, 64
NCORES = 8
NB = B // NCORES          # 4 batches per core
T = NB * PL               # 2308 tokens per core
P = 128
DT = D // P               # 6 contraction tiles
SCALE = float((D // 8) ** -0.5)   # 96**-0.5 (module bug kept faithful)

FB = mybir.dt.bfloat16
F32 = mybir.dt.float32

QS = [128, 128, 128, 128, 65]          # q-subtiles of 577
PCH = [(0, 512), (512, 65)]            # p-chunks of 577
VCH = [(0, 512), (512, 256)]           # V projection chunks of 768


def build_bass(reps=1, in_q="mix", out_q="sp", shared_pools=False,
               bias_eng="dve", vcopy_eng="dve", avs_dt="fb"):
    nc = bacc.Bacc("TRN2", target_bir_lowering=False, debug=False,
                   num_devices=NCORES)

    x_t = nc.dram_tensor("x_t", [D, T], FB, kind="ExternalInput").ap()
    w_qkv_t = nc.dram_tensor("w_qkv_t", [D, 3 * D], FB, kind="ExternalInput").ap()
    w_out_t = nc.dram_tensor("w_out_t", [D, D], FB, kind="ExternalInput").ap()
    b_qk = nc.dram_tensor("b_qk", [P, 12], F32, kind="ExternalInput").ap()
    b_out = nc.dram_tensor("b_out", [P, DT], F32, kind="ExternalInput").ap()
    out_d = nc.dram_tensor("out", [D, T], FB, kind="ExternalOutput").ap()
    out_v = out_d.rearrange("(o p) t -> p o t", p=P)
    xv = x_t.rearrange("(o p) t -> p o t", p=P)
    wv = w_qkv_t.rearrange("(o p) e -> p o e", p=P)

    with tile.TileContext(nc) as tc:
      def emit_rep(singles, btp, atpool, nrm, ypool, pst, pg):
            # per-k input tiles so matmuls start after the first k arrives;
            # x DMA split per batch so batch-0 projections gate only on the
            # first quarter of each k-tile
            # inputs split across both HWDGE queues (SP + ACT) so the two
            # rings stream in parallel. Ramp-critical order: the first ST
            # needs all six q/k weight halves plus batch-0 x, so those lead
            # both queues (4 w-qk on SP, 2 on ACT behind the x chunks);
            # biases and v-weights follow.
            q2 = nc.scalar if in_q == "mix" else nc.sync
            xt, wqkv = [], []
            for k in range(DT):
                wk = singles.tile([P, 3 * D], FB, tag=f"wq{k}", name=f"wq{k}")
                wqkv.append(wk)
                xk = singles.tile([P, T], FB, tag=f"xt{k}", name=f"xt{k}")
                q2.dma_start(xk[:, 0:PL], xv[:, k, 0:PL])
                xt.append(xk)
            for k in range(DT):
                (nc.sync if k < 4 else q2).dma_start(
                    wqkv[k][:, D:3 * D], wv[:, k, D:3 * D])
            bqk = singles.tile([P, 12], F32, tag="bqk")
            q2.dma_start(bqk[:], b_qk)
            bo = singles.tile([P, DT], F32, tag="bo")
            q2.dma_start(bo[:], b_out)
            for k in range(DT):
                (nc.sync if k % 2 == 0 else q2).dma_start(
                    wqkv[k][:, 0:D], wv[:, k, 0:D])
            # out-proj weights: one plain 2D DMA per 128-row block (a single
            # 3D-AP DMA here hits the slow SWDGE descriptor path and parks
            # the SP sequencer for ~50us)
            wo = singles.tile([P, DT, D], FB, tag="wo")
            wov = w_out_t.rearrange("(o p) e -> o p e", p=P)
            for o in range(DT):
                (nc.sync if o % 2 == 0 else q2).dma_start(
                    wo[:, o, :], wov[o])
            for k in range(DT):
                for b in range(1, NB):
                    (nc.sync if (k + b) % 2 == 0 else q2).dma_start(
                        xt[k][:, b * PL:(b + 1) * PL],
                        xv[:, k, b * PL:(b + 1) * PL])

            idt = singles.tile([P, P], FB, tag="idt", name="idt")
            make_identity(nc, idt[:])

            qkt, vbuf, ot, at, on = {}, {}, {}, {}, {}

            def emit_proj_j(b, j):
                # q/k projection for one 128-feature block -> qkT[(b,j)]
                ps = pg.tile([P, D], F32, tag="prj", name="psqk")
                for k in range(DT):
                    for (c0, cw) in PCH:
                        nc.tensor.matmul(
                            ps[:, c0:c0 + cw],
                            lhsT=wqkv[k][:, D + j * P: D + (j + 1) * P],
                            rhs=xt[k][:, b * PL + c0: b * PL + c0 + cw],
                            start=(k == 0), stop=(k == DT - 1),
                            skip_group_check=True)
                qt_tile = btp.tile([P, PL], FB, tag=f"qkt{j}", name=f"qkt{j}")
                beng = nc.gpsimd if bias_eng == "pool" else nc.vector
                beng.tensor_scalar_add(qt_tile[:], ps[:, 0:PL],
                                       bqk[:, j:j + 1])
                qkt[(b, j)] = qt_tile

            def emit_projv(b, tt):
                # V projection for one 128-token block -> vbuf[(b,tt)]
                rows = QS[tt]
                ps = pg.tile([P, D], F32, tag="prj", name="psv")
                for k in range(DT):
                    for (c0, cw) in VCH:
                        nc.tensor.matmul(
                            ps[:rows, c0:c0 + cw],
                            lhsT=xt[k][:, b * PL + tt * P: b * PL + tt * P + rows],
                            rhs=wqkv[k][:, c0:c0 + cw],
                            start=(k == 0), stop=(k == DT - 1),
                            skip_group_check=True)
                vt = btp.tile([P, H, S + 1], FB, tag=f"v{tt}", name=f"v{tt}")
                nc.gpsimd.memset(vt[:, :, S:S + 1], 1.0)
                if vcopy_eng == "act":
                    nc.scalar.copy(
                        vt[:rows, :, 0:S],
                        ps[:rows].rearrange("p (h s) -> p h s", h=H))
                else:
                    nc.vector.tensor_copy(
                        vt[:rows, :, 0:S],
                        ps[:rows].rearrange("p (h s) -> p h s", h=H))
                vbuf[(b, tt)] = vt

            def emit_st(b, j2, qt):
                # scores + exp for one 128-key-token block of a head pair
                rows = QS[qt]
                qs_ = qkt[(b, j2)]
                ks_ = qkt[(b, 6 + j2)]
                if qt == 0:
                    at[(b, j2, 0)] = atpool.tile([P, 5, PL], FB, tag="at",
                                                 name="at0")
                    at[(b, j2, 1)] = atpool.tile([P, 5, PL], FB, tag="at",
                                                 name="at1")
                st0 = pst.tile([P, PL], F32, tag="st", name="st0")
                st1 = pst.tile([P, PL], F32, tag="st", name="st1")
                for (c0, cw) in PCH:
                    nc.tensor.matmul(
                        st0[:rows, c0:c0 + cw],
                        lhsT=ks_[0:64, qt * P: qt * P + rows],
                        rhs=qs_[0:64, c0:c0 + cw],
                        start=True, stop=True,
                        skip_group_check=True)
                    nc.tensor.matmul(
                        st1[:rows, c0:c0 + cw],
                        lhsT=ks_[64:128, qt * P: qt * P + rows],
                        rhs=qs_[64:128, c0:c0 + cw],
                        start=True, stop=True,
                        skip_group_check=True)
                nc.scalar.activation(
                    at[(b, j2, 0)][:rows, qt, :], st0[:rows, :],
                    mybir.ActivationFunctionType.Exp, scale=SCALE)
                nc.scalar.activation(
                    at[(b, j2, 1)][:rows, qt, :], st1[:rows, :],
                    mybir.ActivationFunctionType.Exp, scale=SCALE)

            def emit_avq(b, h):
                # q-major attention-weighted V: O[q, s|1] accumulated over
                # the five key-token tiles into a single-bank PSUM tile
                # [128, 5, 65] (each q-tile's 65-col accumulation group sits
                # inside the bank). The ones column of V lands the softmax
                # denominator at col 64 of each group, so normalization is a
                # per-partition reciprocal + scalar multiply on DVE — no
                # partition broadcast, no odd-head partition-shift DMA.
                j2, half = h // 2, h % 2
                if h == 0:
                    for qt in range(5):
                        on[(b, qt)] = btp.tile([P, H, S], FB, tag=f"on{qt}",
                                               name=f"on{qt}")
                ath = at[(b, j2, half)]
                avq = pg.tile([P, 5, S + 1], F32, tag="prj", name="avq")
                for qt in range(5):
                    qrows = QS[qt]
                    for pt in range(5):
                        prows = QS[pt]
                        nc.tensor.matmul(
                            avq[0:qrows, qt, :],
                            lhsT=ath[:prows, pt, qt * P: qt * P + qrows],
                            rhs=vbuf[(b, pt)][:prows, h, :],
                            start=(pt == 0), stop=(pt == 4),
                            skip_group_check=True)
                rec = nrm.tile([P, 5], F32, tag="rec", name="rec")
                nc.vector.reciprocal(rec[:], avq[:, :, S])
                for qt in range(5):
                    qrows = QS[qt]
                    nc.vector.tensor_scalar_mul(
                        on[(b, qt)][:qrows, h, :],
                        avq[0:qrows, qt, 0:S], rec[:qrows, qt:qt + 1])

            def emit_tpose(b, j2):
                # PE-transpose the head pair's normalized O[q, 128] back to
                # feature-major ot[(b, j2)] for the output projection
                tp = pg.tile([P, PL], FB, tag="prj", name="tp")
                for qt in range(5):
                    qrows = QS[qt]
                    nc.tensor.transpose(
                        tp[:, qt * P: qt * P + qrows],
                        on[(b, qt)][:qrows, 2 * j2: 2 * j2 + 2, :],
                        idt[0:qrows, 0:qrows])
                ot[(b, j2)] = btp.tile([P, PL], FB, tag=f"ot{j2}",
                                       name=f"ot{j2}")
                nc.vector.tensor_copy(ot[(b, j2)][:], tp[:, 0:PL])

            def emit_out(b, m):
                # output projection for one 128-feature block + store
                ps = pg.tile([P, D], F32, tag="prj", name="psy")
                for k in range(DT):
                    for (c0, cw) in PCH:
                        nc.tensor.matmul(
                            ps[:, c0:c0 + cw],
                            lhsT=wo[:, k, m * P:(m + 1) * P],
                            rhs=ot[(b, k)][:, c0:c0 + cw],
                            start=(k == 0), stop=(k == DT - 1),
                            skip_group_check=True)
                ysb = ypool.tile([P, PL], FB, tag="ysb", name="ysb")
                beng = nc.gpsimd if bias_eng == "pool" else nc.vector
                beng.tensor_scalar_add(ysb[:], ps[:, 0:PL], bo[:, m:m + 1])
                oq = nc.scalar if (out_q == "mix" and m % 2 == 1) else nc.sync
                oq.dma_start(out_v[:, m, b * PL:(b + 1) * PL], ysb[:])

            # ---- software-pipelined emission schedule ----
            # Each iteration (b, j2) emits its own batch's NEXT j2-pair of
            # q/k projections, so every batch (including the last) carries
            # its projection filler; batch b+1's first pair rides iter
            # (b, 5). Prologue covers only batch 0's first pair + V.
            emit_proj_j(0, 0)
            emit_proj_j(0, 6)
            for tt in range(5):
                emit_projv(0, tt)

            def proj_filler(b, j2, which):
                # which=0 -> first PROJ slot, which=1 -> second
                if j2 < 5:
                    j = (j2 + 1) if which == 0 else (j2 + 7)
                    emit_proj_j(b, j)
                elif b + 1 < NB:
                    emit_proj_j(b + 1, 0 if which == 0 else 6)

            for b in range(NB):
                for j2 in range(6):
                    # previous head pair to normalize: (b, j2-1), wrapping to
                    # (b-1, 5) at j2=0
                    pb, pj2 = (b, j2 - 1) if j2 > 0 else (b - 1, 5)
                    emit_st(b, j2, 0)
                    # PROJV early so its ACT V-copy's psum wait never
                    # head-of-line blocks the exp stream behind it
                    if b + 1 < NB and j2 < 5:
                        emit_projv(b + 1, j2)
                    emit_st(b, j2, 1)
                    proj_filler(b, j2, 0)
                    emit_st(b, j2, 2)
                    if pb >= 0:
                        emit_avq(pb, 2 * pj2)
                    # OUT rides the ST2->ST3 gap (the q-major AVQ unit alone
                    # is too short to cover the exp cadence); at j2=0 it must
                    # follow TPOSE(b-1,5), which lands this iteration
                    if b > 0 and j2 > 0:
                        emit_out(b - 1, j2)
                    emit_st(b, j2, 3)
                    if pb >= 0:
                        emit_avq(pb, 2 * pj2 + 1)
                    emit_st(b, j2, 4)
                    # PROJ filler before TPOSE: pads ~1.4us so the DVE
                    # normalize-multiplies land before the PE transposes
                    proj_filler(b, j2, 1)
                    if pb >= 0:
                        emit_tpose(pb, pj2)
                    if b > 0 and j2 == 0:
                        emit_out(b - 1, 0)
            # epilogue: last head pair + last batch's output projection.
            # OUT(3,0)'s k=0..4 chunks pad between the final AVQ and TPOSE
            # so the DVE normalize-multiplies land before the transposes;
            # its k=5 chunk (needs ot(3,5) from that transpose) closes after.
            emit_avq(NB - 1, 10)
            emit_avq(NB - 1, 11)
            ps0 = pg.tile([P, D], F32, tag="prj", name="psy")
            for k in range(DT - 1):
                for (c0, cw) in PCH:
                    nc.tensor.matmul(
                        ps0[:, c0:c0 + cw],
                        lhsT=wo[:, k, 0:P],
                        rhs=ot[(NB - 1, k)][:, c0:c0 + cw],
                        start=(k == 0), stop=False,
                        skip_group_check=True)
            emit_tpose(NB - 1, 5)
            for (c0, cw) in PCH:
                nc.tensor.matmul(
                    ps0[:, c0:c0 + cw],
                    lhsT=wo[:, DT - 1, 0:P],
                    rhs=ot[(NB - 1, DT - 1)][:, c0:c0 + cw],
                    start=False, stop=True,
                    skip_group_check=True)
            ysb0 = ypool.tile([P, PL], FB, tag="ysb", name="ysb")
            nc.vector.tensor_scalar_add(ysb0[:], ps0[:, 0:PL], bo[:, 0:1])
            nc.sync.dma_start(out_v[:, 0, (NB - 1) * PL: NB * PL], ysb0[:])
            for m in range(1, DT):
                emit_out(NB - 1, m)

      def open_pools():
        return (tc.tile_pool(name="singles", bufs=1),
                tc.tile_pool(name="bt", bufs=2),
                tc.tile_pool(name="atp", bufs=6),
                tc.tile_pool(name="nrm", bufs=8),
                tc.tile_pool(name="yout", bufs=4),
                tc.tile_pool(name="pst", bufs=2, space="PSUM"),
                tc.tile_pool(name="pg", bufs=2, space="PSUM"))

      import contextlib
      if shared_pools:
          # one pool generation across reps: no inter-rep drain barrier
          with contextlib.ExitStack() as st:
              pools = [st.enter_context(p) for p in open_pools()]
              for _rep in range(reps):
                  emit_rep(*pools)
      else:
          for _rep in range(reps):
              with contextlib.ExitStack() as st:
                  pools = [st.enter_context(p) for p in open_pools()]
                  emit_rep(*pools)

    nc.compile()
    return nc


_NC = None


def _get_nc():
    global _NC
    if _NC is None:
        _NC = build_bass()
    return _NC


def make_in_maps(x, qkv_w, qkv_b, out_w, out_b):
    """Host-side shard + layout prep. Returns per-core input dicts."""
    bf16 = ml_dtypes.bfloat16
    x = np.asarray(x, dtype=np.float32)
    qkv_w = np.asarray(qkv_w, dtype=np.float32)
    qkv_b = np.asarray(qkv_b, dtype=np.float32)
    out_w = np.asarray(out_w, dtype=np.float32)
    out_b = np.asarray(out_b, dtype=np.float32)

    w_qkv_t = np.ascontiguousarray(qkv_w.T).astype(bf16)          # [768, 2304]
    w_out_t = np.ascontiguousarray(out_w.T).astype(bf16)          # [768, 768]
    b_qk = np.ascontiguousarray(qkv_b[D:3 * D].reshape(12, P).T)  # [128, 12]
    # v-bias passes linearly through the output projection (softmax rows sum
    # to 1): fold it into an effective output bias.
    b_eff = out_b + out_w @ qkv_b[0:D]
    b_out = np.ascontiguousarray(b_eff.reshape(DT, P).T)          # [128, 6]

    in_maps = []
    for c in range(NCORES):
        xc = x[c * NB:(c + 1) * NB].reshape(T, D)                 # [2308, 768]
        x_t = np.ascontiguousarray(xc.T).astype(bf16)             # [768, 2308]
        in_maps.append({
            "x_t": x_t,
            "w_qkv_t": w_qkv_t,
            "w_out_t": w_out_t,
            "b_qk": b_qk.astype(np.float32),
            "b_out": b_out.astype(np.float32),
        })
    return in_maps


def assemble_output(results):
    """Per-core 'out' [768, 2308] bf16 -> full [32, 577, 768] f32."""
    y = np.empty((B, PL, D), dtype=np.float32)
    for c in range(NCORES):
        yt = results[c]["out"].astype(np.float32)                 # [768, 2308]
        y[c * NB:(c + 1) * NB] = yt.T.reshape(NB, PL, D)
    return y


def run(x, qkv_w, qkv_b, out_w, out_b, trace=False):
    nc = _get_nc()
    in_maps = make_in_maps(x, qkv_w, qkv_b, out_w, out_b)
    res = run_bass_kernel_spmd(nc, in_maps, core_ids=list(range(NCORES)),
                               trace=trace)
    return assemble_output(res.results), res


def kernel(x, qkv_w, qkv_b, out_w, out_b):
    y, _ = run(x, qkv_w, qkv_b, out_w, out_b)
    return y


# revision 52
# speedup vs baseline: 1.4565x; 1.4565x over previous
"""Trainium2 Bass kernel for nn_Attention (B=32, P=577, D=768, 12 heads).

Strategy: data-parallel over batch — 4 batch elements per core on 8 cores,
zero collectives. Projections and scores run in transposed ([feature,
token]) layouts; attention-weighted V runs q-major and is PE-transposed
back:

  xT [768,2308] --(wqkvT)--> qkT [1536,577/batch] (e on partitions)
                         `--> V   [577/batch,768]  (token on partitions)
  per (b,h): ST = K Q^T   (ST[p,q], K=64, head pairs in PE row halves)
             AT = exp(scale*ST)        (ScalarE, softmax w/o max-subtract:
                                        |scale*S| < ~2, exp is safe)
  O[q, s|1] = AT^T [V|1]  (q-major; ones column of V lands the softmax
             denominator at col 64; all five q-tiles accumulate in a
             single-bank PSUM tile [128, 5, 65])
  O_n = O * (1/denom)     (DVE reciprocal + per-partition scalar multiply —
             no partition broadcast, no partition-shift DMA)
  ot[(b,j2)] = PE-transpose of head-pair O_n  (identity-matmul, bf16 PSUM)
  yT [768,577/batch] = w_outT^T ot + b_eff (v-bias folded into b_eff on host)

Scheduling: the PE executes in emission order, so the emitter software-
pipelines at sub-batch granularity; per (b, j2) iteration the five ST score
units (each gated by the previous exp's PSUM drain) are spaced by
always-ready filler matmuls:

  ST(qt0) PROJV ST(qt1) PROJ ST(qt2) AVQ OUT ST(qt3) AVQ ST(qt4) PROJ TPOSE

(OUT rides the ST2->ST3 gap since the q-major AVQ unit alone is shorter
than the exp cadence — at j2=0 it follows TPOSE(b-1,5) instead; the PROJ
filler precedes TPOSE so the DVE normalize-multiplies land before the PE
transposes read them.)

Every batch (including the last) carries its own next projection pair as
filler, so no iteration is starved. The q-major AV keeps the whole
normalization on DVE with a short PSUM drain, which recycles the shared
PSUM ring quickly — this removed ~160us/rep of real-HW cross-engine stall
versus the O^T formulation with gpsimd partition-broadcast.

Startup: the first ST gates on six q/k weight halves + batch-0 x, so those
lead both HWDGE queues (4 w-qk on SP, 2 on ACT behind the x chunks);
biases, v-weights, out-weights and the remaining x batches follow. Output
is stored bf16 (halves output DMA; converted to f32 on host). Inputs are
bf16; PSUM accumulation is f32.

Measured (axon trn2, differential reps=10 interleaved block-medians):
~239us/exec (rounds 210-292us) vs ~475us for the O^T checkpoint and
~465-495us for the original baseline under equivalent conditions — the
q-major rewrite measured ~164us/rep faster than the checkpoint in matched
A/B, and the PROJ-before-TPOSE tail reorder a further ~70us. TimelineSim
predicts 300us; real HW amplifies cross-engine stall costs, so emission-
order fixes often pay more than the simulator suggests.
"""

import numpy as np
import ml_dtypes

import concourse.bass as bass
import concourse.tile as tile
from concourse import bacc, mybir
from concourse.bass_utils import run_bass_kernel_spmd
from concourse.masks import make_identity

# problem dims (hardcoded per harness contract)
B, PL, D = 32, 577, 768
H, S = 12, 64
NCORES = 8
NB = B // NCORES          # 4 batches per core
T = NB * PL               # 2308 tokens per core
P = 128
DT = D // P               # 6 contraction tiles
SCALE = float((D // 8) ** -0.5)   # 96**-0.5 (module bug kept faithful)

FB = mybir.dt.bfloat16
F32 = mybir.dt.float32

QS = [128, 128, 128, 128, 65]          # q-subtiles of 577
PCH = [(0, 512), (512, 65)]            # p-chunks of 577
VCH = [(0, 512), (512, 256)]           # V projection chunks of 768


def build_bass(reps=1, in_q="mix", out_q="sp", shared_pools=False,
               bias_eng="dve", vcopy_eng="dve", avs_dt="fb"):
    nc = bacc.Bacc("TRN2", target_bir_lowering=False, debug=False,
                   num_devices=NCORES)

    x_t = nc.dram_tensor("x_t", [D, T], FB, kind="ExternalInput").ap()
    w_qkv_t = nc.dram_tensor("w_qkv_t", [D, 3 * D], FB, kind="ExternalInput").ap()
    w_out_t = nc.dram_tensor("w_out_t", [D, D], FB, kind="ExternalInput").ap()
    b_qk = nc.dram_tensor("b_qk", [P, 12], F32, kind="ExternalInput").ap()
    b_out = nc.dram_tensor("b_out", [P, DT], F32, kind="ExternalInput").ap()
    out_d = nc.dram_tensor("out", [D, T], FB, kind="ExternalOutput").ap()
    out_v = out_d.rearrange("(o p) t -> p o t", p=P)
    xv = x_t.rearrange("(o p) t -> p o t", p=P)
    wv = w_qkv_t.rearrange("(o p) e -> p o e", p=P)

    with tile.TileContext(nc) as tc:
      def emit_rep(singles, btp, atpool, nrm, ypool, pst, pg):
            # per-k input tiles so matmuls start after the first k arrives;
            # x DMA split per batch so batch-0 projections gate only on the
            # first quarter of each k-tile
            # inputs split across both HWDGE queues (SP + ACT) so the two
            # rings stream in parallel. Ramp-critical order: the first ST
            # needs all six q/k weight halves plus batch-0 x, so those lead
            # both queues (4 w-qk on SP, 2 on ACT behind the x chunks);
            # biases and v-weights follow.
            q2 = nc.scalar if in_q == "mix" else nc.sync
            xt, wqkv = [], []
            for k in range(DT):
                wk = singles.tile([P, 3 * D], FB, tag=f"wq{k}", name=f"wq{k}")
                wqkv.append(wk)
                xk = singles.tile([P, T], FB, tag=f"xt{k}", name=f"xt{k}")
                q2.dma_start(xk[:, 0:PL], xv[:, k, 0:PL])
                xt.append(xk)
            for k in range(DT):
                (nc.sync if k < 4 else q2).dma_start(
                    wqkv[k][:, D:3 * D], wv[:, k, D:3 * D])
            bqk = singles.tile([P, 12], F32, tag="bqk")
            q2.dma_start(bqk[:], b_qk)
            bo = singles.tile([P, DT], F32, tag="bo")
            q2.dma_start(bo[:], b_out)
            for k in range(DT):
                (nc.sync if k % 2 == 0 else q2).dma_start(
                    wqkv[k][:, 0:D], wv[:, k, 0:D])
            # out-proj weights: one plain 2D DMA per 128-row block (a single
            # 3D-AP DMA here hits the slow SWDGE descriptor path and parks
            # the SP sequencer for ~50us)
            wo = singles.tile([P, DT, D], FB, tag="wo")
            wov = w_out_t.rearrange("(o p) e -> o p e", p=P)
            for o in range(DT):
                (nc.sync if o % 2 == 0 else q2).dma_start(
                    wo[:, o, :], wov[o])
            for k in range(DT):
                for b in range(1, NB):
                    (nc.sync if (k + b) % 2 == 0 else q2).dma_start(
                        xt[k][:, b * PL:(b + 1) * PL],
                        xv[:, k, b * PL:(b + 1) * PL])

            idt = singles.tile([P, P], FB, tag="idt", name="idt")
            make_identity(nc, idt[:])

            qkt, vbuf, ot, at, on = {}, {}, {}, {}, {}

            def emit_proj_j(b, j):
                # q/k projection for one 128-feature block -> qkT[(b,j)]
                ps = pg.tile([P, D], F32, tag="prj", name="psqk")
                for k in range(DT):
                    for (c0, cw) in PCH:
                        nc.tensor.matmul(
                            ps[:, c0:c0 + cw],
                            lhsT=wqkv[k][:, D + j * P: D + (j + 1) * P],
                            rhs=xt[k][:, b * PL + c0: b * PL + c0 + cw],
                            start=(k == 0), stop=(k == DT - 1),
                            skip_group_check=True)
                qt_tile = btp.tile([P, PL], FB, tag=f"qkt{j}", name=f"qkt{j}")
                beng = nc.gpsimd if bias_eng == "pool" else nc.vector
                beng.tensor_scalar_add(qt_tile[:], ps[:, 0:PL],
                                       bqk[:, j:j + 1])
                qkt[(b, j)] = qt_tile

            def emit_projv(b, tt):
                # V projection for one 128-token block -> vbuf[(b,tt)]
                rows = QS[tt]
                ps = pg.tile([P, D], F32, tag="prj", name="psv")
                for k in range(DT):
                    for (c0, cw) in VCH:
                        nc.tensor.matmul(
                            ps[:rows, c0:c0 + cw],
                            lhsT=xt[k][:, b * PL + tt * P: b * PL + tt * P + rows],
                            rhs=wqkv[k][:, c0:c0 + cw],
                            start=(k == 0), stop=(k == DT - 1),
                            skip_group_check=True)
                vt = btp.tile([P, H, S + 1], FB, tag=f"v{tt}", name=f"v{tt}")
                nc.gpsimd.memset(vt[:, :, S:S + 1], 1.0)
                if vcopy_eng == "act":
                    nc.scalar.copy(
                        vt[:rows, :, 0:S],
                        ps[:rows].rearrange("p (h s) -> p h s", h=H))
                else:
                    nc.vector.tensor_copy(
                        vt[:rows, :, 0:S],
                        ps[:rows].rearrange("p (h s) -> p h s", h=H))
                vbuf[(b, tt)] = vt

            def emit_st(b, j2, qt):
                # scores + exp for one 128-key-token block of a head pair
                rows = QS[qt]
                qs_ = qkt[(b, j2)]
                ks_ = qkt[(b, 6 + j2)]
                if qt == 0:
                    at[(b, j2, 0)] = atpool.tile([P, 5, PL], FB, tag="at",
                                                 name="at0")
                    at[(b, j2, 1)] = atpool.tile([P, 5, PL], FB, tag="at",
                                                 name="at1")
                st0 = pst.tile([P, PL], F32, tag="st", name="st0")
                st1 = pst.tile([P, PL], F32, tag="st", name="st1")
                for (c0, cw) in PCH:
                    nc.tensor.matmul(
                        st0[:rows, c0:c0 + cw],
                        lhsT=ks_[0:64, qt * P: qt * P + rows],
                        rhs=qs_[0:64, c0:c0 + cw],
                        start=True, stop=True,
                        skip_group_check=True)
                    nc.tensor.matmul(
                        st1[:rows, c0:c0 + cw],
                        lhsT=ks_[64:128, qt * P: qt * P + rows],
                        rhs=qs_[64:128, c0:c0 + cw],
                        start=True, stop=True,
                        skip_group_check=True)
                nc.scalar.activation(
                    at[(b, j2, 0)][:rows, qt, :], st0[:rows, :],
                    mybir.ActivationFunctionType.Exp, scale=SCALE)
                nc.scalar.activation(
                    at[(b, j2, 1)][:rows, qt, :], st1[:rows, :],
                    mybir.ActivationFunctionType.Exp, scale=SCALE)

            def emit_avq(b, h):
                # q-major attention-weighted V: O[q, s|1] accumulated over
                # the five key-token tiles into a single-bank PSUM tile
                # [128, 5, 65] (each q-tile's 65-col accumulation group sits
                # inside the bank). The ones column of V lands the softmax
                # denominator at col 64 of each group, so normalization is a
                # per-partition reciprocal + scalar multiply on DVE — no
                # partition broadcast, no odd-head partition-shift DMA.
                j2, half = h // 2, h % 2
                if h == 0:
                    for qt in range(5):
                        on[(b, qt)] = btp.tile([P, H, S], FB, tag=f"on{qt}",
                                               name=f"on{qt}")
                ath = at[(b, j2, half)]
                avq = pg.tile([P, 5, S + 1], F32, tag="prj", name="avq")
                for qt in range(5):
                    qrows = QS[qt]
                    for pt in range(5):
                        prows = QS[pt]
                        nc.tensor.matmul(
                            avq[0:qrows, qt, :],
                            lhsT=ath[:prows, pt, qt * P: qt * P + qrows],
                            rhs=vbuf[(b, pt)][:prows, h, :],
                            start=(pt == 0), stop=(pt == 4),
                            skip_group_check=True)
                rec = nrm.tile([P, 5], F32, tag="rec", name="rec")
                nc.vector.reciprocal(rec[:], avq[:, :, S])
                for qt in range(5):
                    qrows = QS[qt]
                    nc.vector.tensor_scalar_mul(
                        on[(b, qt)][:qrows, h, :],
                        avq[0:qrows, qt, 0:S], rec[:qrows, qt:qt + 1])

            def emit_tpose(b, j2):
                # PE-transpose the head pair's normalized O[q, 128] back to
                # feature-major ot[(b, j2)] for the output projection
                tp = pg.tile([P, PL], FB, tag="prj", name="tp")
                for qt in range(5):
                    qrows = QS[qt]
                    nc.tensor.transpose(
                        tp[:, qt * P: qt * P + qrows],
                        on[(b, qt)][:qrows, 2 * j2: 2 * j2 + 2, :],
                        idt[0:qrows, 0:qrows])
                ot[(b, j2)] = btp.tile([P, PL], FB, tag=f"ot{j2}",
                                       name=f"ot{j2}")
                nc.vector.tensor_copy(ot[(b, j2)][:], tp[:, 0:PL])

            def emit_out(b, m):
                # output projection for one 128-feature block + store
                ps = pg.tile([P, D], F32, tag="prj", name="psy")
                for k in range(DT):
                    for (c0, cw) in PCH:
                        nc.tensor.matmul(
                            ps[:, c0:c0 + cw],
                            lhsT=wo[:, k, m * P:(m + 1) * P],
                            rhs=ot[(b, k)][:, c0:c0 + cw],
                            start=(k == 0), stop=(k == DT - 1),
                            skip_group_check=True)
                ysb = ypool.tile([P, PL], FB, tag="ysb", name="ysb")
                beng = nc.gpsimd if bias_eng == "pool" else nc.vector
                beng.tensor_scalar_add(ysb[:], ps[:, 0:PL], bo[:, m:m + 1])
                oq = nc.scalar if (out_q == "mix" and m % 2 == 1) else nc.sync
                oq.dma_start(out_v[:, m, b * PL:(b + 1) * PL], ysb[:])

            # ---- software-pipelined emission schedule ----
            # Each iteration (b, j2) emits its own batch's NEXT j2-pair of
            # q/k projections, so every batch (including the last) carries
            # its projection filler; batch b+1's first pair rides iter
            # (b, 5). Prologue covers only batch 0's first pair + V.
            emit_proj_j(0, 0)
            emit_proj_j(0, 6)
            for tt in range(5):
                emit_projv(0, tt)

            def proj_filler(b, j2, which):
                # which=0 -> first PROJ slot, which=1 -> second
                if j2 < 5:
                    j = (j2 + 1) if which == 0 else (j2 + 7)
                    emit_proj_j(b, j)
                elif b + 1 < NB:
                    emit_proj_j(b + 1, 0 if which == 0 else 6)

            for b in range(NB):
                for j2 in range(6):
                    # previous head pair to normalize: (b, j2-1), wrapping to
                    # (b-1, 5) at j2=0
                    pb, pj2 = (b, j2 - 1) if j2 > 0 else (b - 1, 5)
                    emit_st(b, j2, 0)
                    # PROJV early so its ACT V-copy's psum wait never
                    # head-of-line blocks the exp stream behind it
                    if b + 1 < NB and j2 < 5:
                        emit_projv(b + 1, j2)
                    emit_st(b, j2, 1)
                    proj_filler(b, j2, 0)
                    emit_st(b, j2, 2)
                    if pb >= 0:
                        emit_avq(pb, 2 * pj2)
                    # OUT rides the ST2->ST3 gap (the q-major AVQ unit alone
                    # is too short to cover the exp cadence); at j2=0 it must
                    # follow TPOSE(b-1,5), which lands this iteration
                    if b > 0 and j2 > 0:
                        emit_out(b - 1, j2)
                    emit_st(b, j2, 3)
                    if pb >= 0:
                        emit_avq(pb, 2 * pj2 + 1)
                    emit_st(b, j2, 4)
                    # PROJ filler before TPOSE: pads ~1.4us so the DVE
                    # normalize-multiplies land before the PE transposes
                    proj_filler(b, j2, 1)
                    if pb >= 0:
                        emit_tpose(pb, pj2)
                    if b > 0 and j2 == 0:
                        emit_out(b - 1, 0)
            # epilogue: last head pair + last batch's output projection
            emit_avq(NB - 1, 10)
            emit_avq(NB - 1, 11)
            emit_tpose(NB - 1, 5)
            for m in range(DT):
                emit_out(NB - 1, m)

      def open_pools():
        return (tc.tile_pool(name="singles", bufs=1),
                tc.tile_pool(name="bt", bufs=2),
                tc.tile_pool(name="atp", bufs=6),
                tc.tile_pool(name="nrm", bufs=8),
                tc.tile_pool(name="yout", bufs=4),
                tc.tile_pool(name="pst", bufs=2, space="PSUM"),
                tc.tile_pool(name="pg", bufs=2, space="PSUM"))

      import contextlib
      if shared_pools:
          # one pool generation across reps: no inter-rep drain barrier
          with contextlib.ExitStack() as st:
              pools = [st.enter_context(p) for p in open_pools()]
              for _rep in range(reps):
                  emit_rep(*pools)
      else:
          for _rep in range(reps):
              with contextlib.ExitStack() as st:
                  pools = [st.enter_context(p) for p in open_pools()]
                  emit_rep(*pools)

    nc.compile()
    return nc


_NC = None


def _get_nc():
    global _NC
    if _NC is None:
        _NC = build_bass()
    return _NC


def make_in_maps(x, qkv_w, qkv_b, out_w, out_b):
    """Host-side shard + layout prep. Returns per-core input dicts."""
    bf16 = ml_dtypes.bfloat16
    x = np.asarray(x, dtype=np.float32)
    qkv_w = np.asarray(qkv_w, dtype=np.float32)
    qkv_b = np.asarray(qkv_b, dtype=np.float32)
    out_w = np.asarray(out_w, dtype=np.float32)
    out_b = np.asarray(out_b, dtype=np.float32)

    w_qkv_t = np.ascontiguousarray(qkv_w.T).astype(bf16)          # [768, 2304]
    w_out_t = np.ascontiguousarray(out_w.T).astype(bf16)          # [768, 768]
    b_qk = np.ascontiguousarray(qkv_b[D:3 * D].reshape(12, P).T)  # [128, 12]
    # v-bias passes linearly through the output projection (softmax rows sum
    # to 1): fold it into an effective output bias.
    b_eff = out_b + out_w @ qkv_b[0:D]
    b_out = np.ascontiguousarray(b_eff.reshape(DT, P).T)          # [128, 6]

    in_maps = []
    for c in range(NCORES):
        xc = x[c * NB:(c + 1) * NB].reshape(T, D)                 # [2308, 768]
        x_t = np.ascontiguousarray(xc.T).astype(bf16)             # [768, 2308]
        in_maps.append({
            "x_t": x_t,
            "w_qkv_t": w_qkv_t,
            "w_out_t": w_out_t,
            "b_qk": b_qk.astype(np.float32),
            "b_out": b_out.astype(np.float32),
        })
    return in_maps


def assemble_output(results):
    """Per-core 'out' [768, 2308] bf16 -> full [32, 577, 768] f32."""
    y = np.empty((B, PL, D), dtype=np.float32)
    for c in range(NCORES):
        yt = results[c]["out"].astype(np.float32)                 # [768, 2308]
        y[c * NB:(c + 1) * NB] = yt.T.reshape(NB, PL, D)
    return y


def run(x, qkv_w, qkv_b, out_w, out_b, trace=False):
    nc = _get_nc()
    in_maps = make_in_maps(x, qkv_w, qkv_b, out_w, out_b)
    res = run_bass_kernel_spmd(nc, in_maps, core_ids=list(range(NCORES)),
                               trace=trace)
    return assemble_output(res.results), res


def kernel(x, qkv_w, qkv_b, out_w, out_b):
    y, _ = run(x, qkv_w, qkv_b, out_w, out_b)
    return y
